# revision 1
# baseline (speedup 1.0000x reference)
"""Trainium2 Bass kernel v3 for nn_CHGANSimplified (sparse graph attention).

Math (per batch b, time t):
  enh = x + type_embed[parity(n)]
  Q/K/V = enh @ W*.T + b*          (4 heads, head dim 32)
  S_h = (Q_h K_h^T)/sqrt(32) + edge_bias ; masked where adj==0 & ~eye
  out = LN(concat_h(softmax(S_h) V_h) @ Wo.T + bo + x)

v2 vs v1:
  - type-embed folded host-side: qta=(Wq@ta+bq)*s, kta=Wk@ta+bk,
    vtab=(ta.T@Wv.T+bv) -> no enh tile, xT ships bf16 (half the DMA)
  - AV accumulates all 8 q-blocks of a head into one PSUM tile; the
    softmax normalize is 2 batched DVE ops per (pair,head) instead of 24
  - LayerNorm rstd via DVE quake-rsqrt; ACT runs Exp only (no table swaps)
  - emission-order software pipelining across (pair, head) slots: AV of
    slot s-1 is interleaved into the score matmuls of slot s; next pair's
    QKV is emitted during head 2/3; epilogue during next pair's head 1
"""

import os
import sys

sys.path.insert(0, "/opt/trn_rl_repo")

from contextlib import ExitStack

import ml_dtypes
import numpy as np

import concourse.bass as bass
import concourse.tile as tile
from concourse import bacc, mybir
from concourse.bass_utils import run_bass_kernel_spmd

B, N, T, D, H, DH = 2, 1024, 12, 128, 4, 32
NCORES = 8
PAIRS = [(b, t) for b in range(B) for t in range(T)]
PER_CORE = len(PAIRS) // NCORES  # 3
EPS = 1e-5
NTILE = N // 128  # 8

MM_DT, MM_NP = mybir.dt.bfloat16, ml_dtypes.bfloat16
F32 = mybir.dt.float32
I32 = mybir.dt.int32
AF = mybir.ActivationFunctionType
ALU = mybir.AluOpType

# mask routing per m-tile: m < INJ -> PE additive inject;
# next GPS -> Pool multiplicative; rest -> DVE multiplicative
INJ = int(os.environ.get("BASSK_INJ", "4"))
GPS = int(os.environ.get("BASSK_GPS", "2"))

QMAGIC = 0x5F3759DF

LAST_RESULTS = None  # BassKernelResults of the most recent run (for test.py)


def _build_nc(ln_trivial: bool):
    nc = bacc.Bacc()

    xT_d = nc.dram_tensor("xt", [PER_CORE, 128, N], MM_DT, kind="ExternalInput")
    xpb_d = nc.dram_tensor("xpb", [PER_CORE, N, D], F32, kind="ExternalInput")
    wq_d = nc.dram_tensor("wq", [D, D], MM_DT, kind="ExternalInput")
    wk_d = nc.dram_tensor("wk", [D, D], MM_DT, kind="ExternalInput")
    wv_d = nc.dram_tensor("wv", [D, D], MM_DT, kind="ExternalInput")
    wo_d = nc.dram_tensor("wo", [D, D], MM_DT, kind="ExternalInput")
    qta_d = nc.dram_tensor("qta", [D, N], MM_DT, kind="ExternalInput")
    kta_d = nc.dram_tensor("kta", [D, N], MM_DT, kind="ExternalInput")
    vtab_d = nc.dram_tensor("vtab", [128, D], F32, kind="ExternalInput")
    lng_d = nc.dram_tensor("lng", [128, D], F32, kind="ExternalInput")
    lnb_d = nc.dram_tensor("lnb", [128, D], F32, kind="ExternalInput")
    id_d = nc.dram_tensor("ident", [128, 128], MM_DT, kind="ExternalInput")
    # mask, transposed (m, nq): additive (PE psum-injection) + multiplicative
    # (DVE/Pool post-exp) variants; each m-tile loads from one of them
    maska_d = nc.dram_tensor("maska", [N, N], MM_DT, kind="ExternalInput")
    maskm_d = nc.dram_tensor("maskm", [N, N], MM_DT, kind="ExternalInput")
    out_d = nc.dram_tensor("out", [PER_CORE, N, D], F32, kind="ExternalOutput")

    with tile.TileContext(nc) as tc, ExitStack() as ctx:
        const = ctx.enter_context(tc.tile_pool(name="const", bufs=1))
        work = ctx.enter_context(tc.tile_pool(name="work", bufs=2))
        expp = ctx.enter_context(tc.tile_pool(name="expp", bufs=16))
        pst = ctx.enter_context(tc.tile_pool(name="pst", bufs=2, space="PSUM"))
        pav = ctx.enter_context(tc.tile_pool(name="pav", bufs=2, space="PSUM"))
        ppv = ctx.enter_context(tc.tile_pool(name="ppv", bufs=2, space="PSUM"))

        # ---- constants ----
        # SP (sync) queue: weights/consts gating the first scores; Pool
        # queue: x inputs + masks. Runs the two prep streams in parallel.
        wq_sb = const.tile([D, D], MM_DT)
        nc.sync.dma_start(wq_sb, wq_d[:, :])
        wk_sb = const.tile([D, D], MM_DT)
        nc.sync.dma_start(wk_sb, wk_d[:, :])
        qta_sb = const.tile([D, N], MM_DT)
        nc.sync.dma_start(qta_sb, qta_d[:, :])
        kta_sb = const.tile([D, N], MM_DT)
        nc.sync.dma_start(kta_sb, kta_d[:, :])

        # pre-load the Exp table while DMAs are in flight so the first real
        # exp doesn't pay the table switch
        tiny = const.tile([128, 1], F32)
        nc.vector.memset(tiny, 0.0)
        tiny2 = const.tile([128, 1], MM_DT)
        nc.scalar.activation(tiny2, tiny, AF.Exp)

        mask_sb = [None] * NTILE
        maska_sb = [None] * NTILE  # additive variant (drain slot injects all)
        def load_mask(m):
            mt = const.tile([128, N], MM_DT, name=f"mask{m}", tag=f"mask{m}")
            src = maska_d if m < INJ else maskm_d
            nc.gpsimd.dma_start(mt, src[m * 128 : (m + 1) * 128, :])
            mask_sb[m] = mt
            if m < INJ:
                maska_sb[m] = mt

        load_mask(0)

        def load_x(it):
            xT_sb = work.tile([128, N], MM_DT, name=f"xT{it}", tag="xT")
            nc.gpsimd.dma_start(xT_sb, xT_d[it])
            xpb_sb = work.tile([128, NTILE, D], F32, name=f"xpb{it}", tag="xpb")
            nc.gpsimd.dma_start(xpb_sb, xpb_d[it].rearrange("(q p) d -> p q d", p=128))
            return xT_sb, xpb_sb

        x_sb = {0: load_x(0)}
        for m in range(1, NTILE):
            load_mask(m)

        wv_sb = const.tile([D, D], MM_DT)
        nc.sync.dma_start(wv_sb, wv_d[:, :])
        vtab_sb = const.tile([128, D], F32)
        nc.sync.dma_start(vtab_sb, vtab_d[:, :])
        wo_sb = const.tile([D, D], MM_DT)
        nc.sync.dma_start(wo_sb, wo_d[:, :])
        id_sb = const.tile([128, 128], MM_DT)
        nc.sync.dma_start(id_sb, id_d[:, :])
        lng_sb = const.tile([128, D], F32)
        nc.sync.dma_start(lng_sb, lng_d[:, :])
        lnb_sb = const.tile([128, D], F32)
        nc.sync.dma_start(lnb_sb, lnb_d[:, :])
        for m in range(INJ, NTILE):
            mt = const.tile([128, N], MM_DT, name=f"maskA{m}", tag=f"maskA{m}")
            nc.sync.dma_start(mt, maska_d[m * 128 : (m + 1) * 128, :])
            maska_sb[m] = mt
        half_sb = const.tile([128, 1], F32)
        nc.vector.memset(half_sb, 0.5)
        c32_sb = const.tile([128, 1], F32)
        nc.vector.memset(c32_sb, 1.5)
        magic_sb = const.tile([128, 1], I32)
        nc.vector.memset(magic_sb, QMAGIC)

        # PE p-state warmup: harmless matmuls on a zeroed tile so the clock
        # ramp (full speed after 3us busy) is done before the real QK work.
        warm_sb = const.tile([128, 512], MM_DT)
        nc.vector.memset(warm_sb, 0.0)
        wps = pav.tile([16, 512], F32, name="warm", tag="av")
        for _ in range(6):
            nc.tensor.matmul(wps, warm_sb[:, 0:16], warm_sb, start=True, stop=True)

        # ---- per-pair state (filled by emit_* helpers) ----
        qk = {}  # it -> (qt_lo, qt_hi, kt_lo, kt_hi)
        vaugs = {}  # it -> [8 x (128, H, DH+1) tiles]
        es_all = {}  # (it, h) -> [8 x es tiles]
        av_all = {}  # (it, h) -> psum tile (128, NTILE, DH+1)
        onat = {}  # it -> (128, NTILE, D) bf16

        def emit_qk(it):
            xT_sb, _ = x_sb[it]
            tiles = []
            for nm, w_sb, ta2 in (("q", wq_sb, qta_sb), ("k", wk_sb, kta_sb)):
                lo = work.tile([64, N], MM_DT, name=f"{nm}tl{it}", tag=f"{nm}tl")
                hi = work.tile([64, N], MM_DT, name=f"{nm}th{it}", tag=f"{nm}th")
                for j in range(2):
                    js = slice(j * 512, (j + 1) * 512)
                    ps = ppv.tile([128, 512], F32, name=f"ps{nm}{it}_{j}", tag="pv")
                    nc.tensor.matmul(ps, w_sb, xT_sb[:, js], start=True, stop=True)
                    nc.vector.tensor_add(lo[:, js], ps[0:64, :], ta2[0:64, js])
                    nc.vector.tensor_add(hi[:, js], ps[64:128, :], ta2[64:128, js])
                tiles += [lo, hi]
            qk[it] = tuple(tiles)

        def emit_v(it):
            xT_sb, _ = x_sb[it]
            va = []
            for m in range(NTILE):
                vps = ppv.tile([128, D], F32, name=f"vps{it}_{m}", tag="pv")
                nc.tensor.matmul(
                    vps, xT_sb[:, m * 128 : (m + 1) * 128], wv_sb, start=True, stop=True
                )
                vt = work.tile(
                    [128, H, DH + 1], MM_DT, name=f"vaug{it}_{m}", tag=f"vaug{m}"
                )
                nc.gpsimd.memset(vt[:, :, DH : DH + 1], 1.0)
                nc.vector.tensor_add(
                    vt[:, :, 0:DH],
                    vps.rearrange("p (h d) -> p h d", h=H),
                    vtab_sb.rearrange("p (h d) -> p h d", h=H),
                )
                va.append(vt)
            vaugs[it] = va

        def emit_score_m(it, h, m, inject_all=False):
            """scores + exp + mask for one (pair, head, m-tile)."""
            qt_lo, qt_hi, kt_lo, kt_hi = qk[it]
            qt_t = qt_lo if h < 2 else qt_hi
            kt_t = kt_lo if h < 2 else kt_hi
            po = 32 * (h % 2)
            st = pst.tile([128, N], F32, name=f"st{it}_{h}_{m}", tag="st")
            inject = m < INJ or inject_all
            if inject:
                for j in range(2):
                    nc.tensor.matmul(
                        st[:, j * 512 : (j + 1) * 512],
                        id_sb,
                        maska_sb[m][:, j * 512 : (j + 1) * 512],
                        start=True,
                        stop=False,
                    )
            for j in range(2):
                nc.tensor.matmul(
                    st[:, j * 512 : (j + 1) * 512],
                    kt_t[po : po + 32, m * 128 : (m + 1) * 128],
                    qt_t[po : po + 32, j * 512 : (j + 1) * 512],
                    start=not inject,
                    stop=True,
                )
            e = expp.tile([128, N], MM_DT, name=f"e{it}_{h}_{m}", tag="expst")
            nc.scalar.activation(e, st, AF.Exp)
            if not inject:
                if m < INJ + GPS:
                    nc.gpsimd.tensor_mul(e, e, mask_sb[m])
                else:
                    nc.vector.tensor_mul(e, e, mask_sb[m])
            return e

        def emit_av_chunk(it, h, q):
            """8 accumulating AV matmuls for q-block q of (pair, head)."""
            if q == 0:
                av_all[(it, h)] = pav.tile(
                    [128, NTILE, DH + 1], F32, name=f"av{it}_{h}", tag="av"
                )
            av = av_all[(it, h)]
            es = es_all[(it, h)]
            va = vaugs[it]
            for m in range(NTILE):
                nc.tensor.matmul(
                    av[:, q, :],
                    es[m][:, q * 128 : (q + 1) * 128],
                    va[m][:, h, :],
                    start=(m == 0),
                    stop=(m == NTILE - 1),
                )

        def emit_norm(it, h):
            """batched reciprocal + normalize for slot (it, h)."""
            if h == 0:
                onat[it] = work.tile([128, NTILE, D], MM_DT, name=f"on{it}", tag="onat")
            av = av_all[(it, h)]
            rec = work.tile([128, NTILE], F32, name=f"rec{it}_{h}", tag="rec", bufs=4)
            nc.vector.reciprocal(rec, av[:, :, DH])
            nc.vector.tensor_mul(
                onat[it][:, :, h * DH : (h + 1) * DH],
                av[:, :, 0:DH],
                rec[:, :, None].to_broadcast((128, NTILE, DH)),
            )

        def emit_epilogue(it):
            _, xpb_sb = x_sb[it]
            ot = work.tile([128, N], MM_DT, name=f"ot{it}", tag="ot")
            for q in range(NTILE):
                tp = ppv.tile([128, 128], MM_DT, name=f"tp{it}_{q}", tag="pv")
                nc.tensor.transpose(tp, onat[it][:, q, :], id_sb)
                nc.vector.tensor_copy(ot[:, q * 128 : (q + 1) * 128], tp)
            y = work.tile([128, NTILE, D], F32, name=f"y{it}", tag="y")
            mv = work.tile([128, NTILE, 2], F32, name=f"mv{it}", tag="mv")
            for q in range(NTILE):
                op = ppv.tile([128, D], F32, name=f"op{it}_{q}", tag="pv")
                nc.tensor.matmul(
                    op, ot[:, q * 128 : (q + 1) * 128], wo_sb, start=True, stop=True
                )
                nc.vector.tensor_add(y[:, q, :], op, xpb_sb[:, q, :])
                st6 = work.tile([128, 6], F32, name=f"st6{it}_{q}", tag="st6", bufs=8)
                nc.vector.bn_stats(st6, y[:, q, :])
                nc.vector.bn_aggr(mv[:, q, :], st6)
            # rstd = 1/sqrt(var+eps) via quake + 2 Newton steps (DVE only)
            ve = work.tile([128, NTILE], F32, name=f"ve{it}", tag="ve")
            nc.vector.tensor_scalar_add(ve, mv[:, :, 1], EPS)
            vh = work.tile([128, NTILE], F32, name=f"vh{it}", tag="vh")
            nc.vector.tensor_scalar_mul(vh, ve, half_sb[:, 0:1])
            yq = work.tile([128, NTILE], F32, name=f"yq{it}", tag="yq")
            yqi = yq.bitcast(I32)
            nc.vector.tensor_scalar(yqi, ve.bitcast(I32), 1, None, ALU.logical_shift_right)
            nc.vector.tensor_tensor(
                yqi, magic_sb[:, 0:1].to_broadcast((128, NTILE)).bitcast(I32), yqi,
                ALU.subtract,
            )
            t1 = work.tile([128, NTILE], F32, name=f"t1{it}", tag="t1")
            t2 = work.tile([128, NTILE], F32, name=f"t2{it}", tag="t2")
            for _ in range(2):
                nc.vector.tensor_tensor(t1, yq, yq, ALU.mult)
                nc.vector.tensor_tensor(t2, vh, t1, ALU.mult)
                nc.vector.tensor_tensor(
                    t1, c32_sb[:, 0:1].to_broadcast((128, NTILE)), t2, ALU.subtract
                )
                nc.vector.tensor_tensor(yq, yq, t1, ALU.mult)
            oall = work.tile([128, NTILE, D], F32, name=f"oall{it}", tag="oall")
            for q in range(NTILE):
                if ln_trivial:
                    nc.vector.tensor_scalar(
                        oall[:, q, :], y[:, q, :], mv[:, q, 0:1], yq[:, q : q + 1],
                        op0=ALU.subtract, op1=ALU.mult,
                    )
                else:
                    z = work.tile([128, D], F32, name=f"z{it}_{q}", tag="z", bufs=4)
                    nc.vector.tensor_scalar(
                        z, y[:, q, :], mv[:, q, 0:1], yq[:, q : q + 1],
                        op0=ALU.subtract, op1=ALU.mult,
                    )
                    nc.vector.tensor_mul(z, z, lng_sb)
                    nc.vector.tensor_add(oall[:, q, :], z, lnb_sb)
                out_dst = out_d[it].rearrange("(q p) d -> p q d", p=128)
                if q == NTILE // 2 - 1:
                    nc.sync.dma_start(out_dst[:, 0 : NTILE // 2], oall[:, 0 : NTILE // 2])
                elif q == NTILE - 1:
                    nc.sync.dma_start(
                        out_dst[:, NTILE // 2 :], oall[:, NTILE // 2 :]
                    )

        # ---- software-pipelined emission over 12 (pair, head) slots ----
        SLOTS = [(it, h) for it in range(PER_CORE) for h in range(H)]
        LASTS = len(SLOTS) - 1
        emit_qk(0)
        for s, (it, h) in enumerate(SLOTS):
            prev = SLOTS[s - 1] if s > 0 else None
            es_all[(it, h)] = es = []
            for m in range(NTILE):
                es.append(emit_score_m(it, h, m, inject_all=(s == LASTS)))
                if prev is not None:
                    emit_av_chunk(*prev, q=m)
            if s == LASTS:
                for q in range(NTILE):
                    emit_av_chunk(it, h, q)
            if prev is not None:
                emit_norm(*prev)
                es_all.pop(prev)
            if s == 0:
                emit_v(0)
            if h == 1 and it + 1 < PER_CORE:
                x_sb[it + 1] = load_x(it + 1)
            if h == 2 and it + 1 < PER_CORE:
                emit_qk(it + 1)
                emit_v(it + 1)
            if h == 1 and it > 0:
                emit_epilogue(it - 1)
        last = SLOTS[-1]
        emit_norm(*last)
        emit_epilogue(PER_CORE - 1)

    nc.compile()
    return nc


_nc_cache = {}


def _get_nc(ln_trivial=True):
    key = (ln_trivial, INJ, GPS)
    if key not in _nc_cache:
        _nc_cache[key] = _build_nc(ln_trivial)
    return _nc_cache[key]


def make_inputs(
    node_features, adj_mx, node_type_embed, Wq, bq, Wk, bk, Wv, bv,
    edge_bias, Wo, bo, ln_g, ln_b,
):
    """Host-side prep: returns (in_maps, ln_trivial)."""
    nf = np.asarray(node_features, np.float32)
    adj = np.asarray(adj_mx)
    nte = np.asarray(node_type_embed, np.float32)
    Wq = np.asarray(Wq, np.float32)
    Wk = np.asarray(Wk, np.float32)
    Wv = np.asarray(Wv, np.float32)
    Wo = np.asarray(Wo, np.float32)
    bq = np.asarray(bq, np.float32)
    bk = np.asarray(bk, np.float32)
    bv = np.asarray(bv, np.float32)
    bo = np.asarray(bo, np.float32)
    edge_bias = np.asarray(edge_bias, np.float32)
    ln_g = np.asarray(ln_g, np.float32)
    ln_b = np.asarray(ln_b, np.float32)

    scale = 1.0 / np.sqrt(DH)
    types = 1 - (np.arange(N) % 2)
    ta = np.ascontiguousarray(nte[types].T)  # (D, N)
    keep = np.maximum(adj.astype(np.float32), np.eye(N, dtype=np.float32))
    maskmul = np.ascontiguousarray((np.exp(edge_bias) * keep).T)  # (m, nq)
    maskadd = np.ascontiguousarray((edge_bias + (keep - 1.0) * 1e30).T)
    qta = (Wq @ ta + bq[:, None]) * scale  # (D, N)
    kta = Wk @ ta + bk[:, None]
    vtab = nte @ Wv.T  # (2, D)
    vtab = vtab[types[:128]] + bv  # (128, D) by partition parity

    ln_trivial = bool(np.all(ln_g == 1.0) and np.all(ln_b == 0.0))

    shared = {
        "wq": np.ascontiguousarray(Wq.T * scale).astype(MM_NP),
        "wk": np.ascontiguousarray(Wk.T).astype(MM_NP),
        "wv": np.ascontiguousarray(Wv.T).astype(MM_NP),
        "wo": np.ascontiguousarray(Wo.T).astype(MM_NP),
        "qta": np.ascontiguousarray(qta).astype(MM_NP),
        "kta": np.ascontiguousarray(kta).astype(MM_NP),
        "vtab": np.ascontiguousarray(vtab),
        "lng": np.ascontiguousarray(np.broadcast_to(ln_g, (128, D))),
        "lnb": np.ascontiguousarray(np.broadcast_to(ln_b, (128, D))),
        "ident": np.eye(128, dtype=MM_NP),
        "maska": maskadd.astype(MM_NP),
        "maskm": maskmul.astype(MM_NP),
    }
    in_maps = []
    for c in range(NCORES):
        pairs = PAIRS[c * PER_CORE : (c + 1) * PER_CORE]
        xT = np.stack(
            [np.ascontiguousarray(nf[b, :, t, :].T).astype(MM_NP) for (b, t) in pairs]
        )
        xpb = np.stack([nf[b, :, t, :] + bo for (b, t) in pairs])
        in_maps.append({**shared, "xt": xT, "xpb": xpb})
    return in_maps, ln_trivial


def kernel(
    node_features, adj_mx, node_type_embed, Wq, bq, Wk, bk, Wv, bv,
    edge_bias, Wo, bo, ln_g, ln_b,
):
    global LAST_RESULTS
    in_maps, ln_trivial = make_inputs(
        node_features, adj_mx, node_type_embed, Wq, bq, Wk, bk, Wv, bv,
        edge_bias, Wo, bo, ln_g, ln_b,
    )
    nc = _get_nc(ln_trivial)
    res = run_bass_kernel_spmd(
        nc,
        in_maps,
        core_ids=list(range(NCORES)),
        trace=bool(int(os.environ.get("BASSK_TRACE", "0"))),
    )
    LAST_RESULTS = res

    out = np.empty((B, N, T, D), np.float32)
    for c in range(NCORES):
        pairs = PAIRS[c * PER_CORE : (c + 1) * PER_CORE]
        for i, (b, t) in enumerate(pairs):
            out[b, :, t, :] = res.results[c]["out"][i]
    return out

